# revision 14
# baseline (speedup 1.0000x reference)
"""LIF Conv2d + STDP kernel for 8 Trainium2 NeuronCores — v2.

Sharding: data-parallel over batch (B=8, one element per core); per-step
STDP weight-gradient correlations are AllGathered (bf16) and summed
locally before the replicated weight update.

Key structures (per core):
  - x_pre state x3f [96, 2176] f32: rows 48h+16kw+c (h = l-half), col
    j <-> l = 2048h + j - 64 (64-col halo both sides), value
    x_pre(c, l + kw - 1) with zero w-borders/halo.  Split each step into
    xr = round_f32r(x3f), xl = round_f32r(x3f - xr) so matmuls run in
    single-pass f32r (1 cyc/row) at fp32-class accuracy.
  - conv i_syn = Wr@xr + Wr@xl (early, W-stationary [96,512] chunks,
    stale-rounded weights Wrs) + Wfix@xr (late, flat [32,512] fold
    chunks) where Wfix = round_f32r(W_new - Wrs): mathematically
    W_new @ x up to ~1e-8 — the late pass doubles as the collective-
    hiding correction, so the AllGather overlaps the early conv.
  - LIF on the fold layout [128, 1024] (partition 32g+o, col 512d+r,
    l = 512(4d+g)+r).
  - dW path in bf16, l-major: S3l/P3 [128, 32, 48] via PE transposes;
    spikes/post-trace in the interleaved YI layout [128, 65, 32]
    (YI[p, s, :] = Y(64(s-1)+p)): odd s = plain 128-chunk transposes,
    even s = two bulk cross-partition shifted copies.  dW matmul k
    contracts 128 l with rhs = YI[:, 2k:2k+3, :] — the three adjacent
    slots ARE the kh shifts; no shift-DMA traffic at all.
"""

import numpy as np

T, B, C_IN, H, W_IN = 32, 8, 16, 64, 64
C_OUT, KH, KW = 32, 3, 3
L = H * W_IN  # 4096
XW = 2176  # 2048 + 2*64 halo
BETA_M = float(np.exp(-1.0 / 20.0))
BETA_S = float(np.exp(-1.0 / 5.0))
BETA_PRE = float(np.exp(-1.0 / 20.0))
BETA_POST = float(np.exp(-1.0 / 20.0))
V_TH = 1.0
T_REF = 2.0
ETA = 5e-4
NORM = float(B * L)
N_CORES = 8


def _patch_tile_drain():
    """walrus in this build rejects >1 sync wait on a CTRL-class (drain)
    instruction; spread the final tile drain's waits across nops."""
    import concourse.tile as tile
    import concourse.mybir as mybir
    from concourse.vector_clock import ScopedClock

    if getattr(tile.TileContext, "_drain_patched", False):
        return

    def _drain_and_barrier(self, tick_clock, wait_clock):
        nc = self.nc
        drain_inst = nc.sync.drain()
        wait_clock.add_sem_waits(
            drain_inst.ins, ScopedClock({None: tick_clock.global_clock})
        )
        si = drain_inst.ins.sync_info
        waits = list(si.on_wait or [])
        if len(waits) > 1:
            si.on_wait = waits[:1]
            for i in range(1, len(waits)):
                nop = nc.sync.nop(nofuse=True)
                nop.ins.sync_info = mybir.SyncInfo(
                    on_wait=waits[i : i + 1], on_update=[]
                )
        nc.all_engine_barrier()
        assert self.sems is not None
        popped = nc._tile_sem_poison_stack.pop()
        assert popped is self._sem_poison
        nc.clear_and_free_semaphores(list(self.sems.allocated().values()))
        nc.all_engine_barrier()

    tile.TileContext._drain_and_barrier = _drain_and_barrier
    tile.TileContext._drain_patched = True


def _split_sync_waits(nc):
    """This walrus build accepts only ONE sync-wait slot per instruction.
    Move extra waits onto injected same-engine nops placed just before."""
    import concourse.mybir as mybir

    n = 0
    for f in nc.m.functions:
        for bb in f.blocks:
            new_insts = []
            for inst in bb.instructions:
                si = inst.sync_info
                waits = list(si.on_wait or []) if si else []
                if len(waits) > 1:
                    for w in waits[:-1]:
                        n += 1
                        nop = mybir.InstNoOp(
                            name=f"I-wsplit-{n}", engine=inst.engine,
                            ins=[], outs=[], bass_nofuse=True,
                            sync_info=mybir.SyncInfo(on_wait=[w], on_update=[]),
                        )
                        new_insts.append(nop)
                    si.on_wait = waits[-1:]
                new_insts.append(inst)
            bb.instructions = new_insts
    return n


_NC_CACHE = {}


def _build(n_steps):
    import concourse.bass as bass
    import concourse.mybir as mybir
    import concourse.tile as tile

    _patch_tile_drain()
    f32 = mybir.dt.float32
    f32r = mybir.dt.float32r
    bf16 = mybir.dt.bfloat16
    u8 = mybir.dt.uint8
    op = mybir.AluOpType

    nc = bass.Bass("TRN2", target_bir_lowering=False, debug=False,
                   num_devices=N_CORES)

    S_d = nc.dram_tensor("S", [T, C_IN, H, W_IN], f32, kind="ExternalInput")
    W_d = nc.dram_tensor("Wk", [48, 96], f32, kind="ExternalInput")
    spk_d = nc.dram_tensor("spk_out", [T, C_OUT, L], u8, kind="ExternalOutput")
    v_d = nc.dram_tensor("v_out", [T, C_OUT, L], f32, kind="ExternalOutput")
    i_d = nc.dram_tensor("i_out", [T, C_OUT, L], f32, kind="ExternalOutput")

    cc_in = [nc.dram_tensor(f"cc_in_{t}", [128, 96], bf16)
             for t in range(n_steps - 1)]
    cc_out = [
        nc.dram_tensor(f"cc_out_{t}", [N_CORES, 128, 96], bf16,
                       addr_space="Shared")
        for t in range(n_steps - 1)
    ]

    id48 = nc.inline_tensor(np.eye(48, dtype=np.float32), "id48")
    id128 = nc.inline_tensor(np.eye(128, dtype=np.float32), "id128")

    eta_n = ETA / NORM

    with tile.TileContext(nc) as tc:
        with (
            tc.tile_pool(name="state", bufs=1) as st,
            tc.tile_pool(name="io", bufs=2) as io,
            tc.tile_pool(name="pse", bufs=2, space=bass.MemorySpace.PSUM) as pse,
            tc.tile_pool(name="psl", bufs=2, space=bass.MemorySpace.PSUM) as psl,
            tc.tile_pool(name="pst", bufs=2, space=bass.MemorySpace.PSUM) as pst,
            tc.tile_pool(name="psw", bufs=1, space=bass.MemorySpace.PSUM) as psw,
        ):
            # ---------------- persistent state ----------------
            x3fA = st.tile([112, XW], f32, tag="x3fA")
            x3fB = st.tile([112, XW], f32, tag="x3fB")
            xr = st.tile([112, XW], f32r, tag="xr")
            xl = st.tile([112, XW], f32r, tag="xl")
            S3c = st.tile([112, XW], f32, tag="S3c")
            S3cb = st.tile([112, XW], bf16, tag="S3cb")
            Wk = st.tile([48, 96], f32, tag="Wk")
            Wrs = st.tile([112, 96], f32r, tag="Wrs")   # stale-rounded W
            Wfix = st.tile([112, 96], f32r, tag="Wfix")
            v = st.tile([128, 1024], f32, tag="v")
            ref = st.tile([128, 1024], f32, tag="ref")
            P3A = st.tile([128, 32, 48], bf16, tag="P3A")
            P3B = st.tile([128, 32, 48], bf16, tag="P3B")
            S3l = st.tile([128, 32, 48], bf16, tag="S3l")
            YIs = st.tile([128, 65, 32], bf16, tag="YIs")
            YIpA = st.tile([128, 65, 32], bf16, tag="YIpA")
            YIpB = st.tile([128, 65, 32], bf16, tag="YIpB")
            zero96 = st.tile([48, 96], f32, tag="zero96")
            i48b = st.tile([112, 48], bf16, tag="i48b")
            i128b = st.tile([128, 128], bf16, tag="i128b")
            i48f = st.tile([48, 48], f32, tag="i48f")
            i128f = st.tile([128, 128], f32, tag="i128f")

            nc.sync.dma_start(Wk[:], W_d[:])
            nc.sync.dma_start(i48f[:], id48[:])
            nc.sync.dma_start(i128f[:], id128[:])
            nc.vector.tensor_copy(i48b[0:48, :], i48f[:])
            nc.vector.tensor_copy(i48b[64:112, :], i48f[:])
            nc.vector.tensor_copy(i128b[:], i128f[:])
            nc.vector.memset(x3fA[:], 0.0)
            nc.vector.memset(zero96[:], 0.0)
            nc.vector.memset(S3c[:], 0.0)
            nc.vector.memset(v[:], 0.0)
            nc.vector.memset(ref[:], 0.0)
            nc.vector.memset(P3A[:], 0.0)
            nc.vector.memset(YIs[:], 0.0)
            nc.vector.memset(YIpA[:], 0.0)
            nc.vector.scalar_tensor_tensor(
                Wrs[0:48, :], Wk[:], 1.0, zero96[:], op.mult, op.add)
            nc.vector.scalar_tensor_tensor(
                Wrs[64:112, :], Wk[:], 1.0, zero96[:], op.mult, op.add)

            S_hw = S_d.ap().rearrange("t c h w -> t c (h w)")

            for t in range(n_steps):
                # ====== S(t) load: 6 DMAs into S3c [96, XW] ======
                # col j <-> l = 2048h + j - 64; cell jc = j//64 <-> hrow =
                # jc - 1 + 32h (the 64-col halo is exactly one w-cell).
                s3v = S3c[:].rearrange("p (jc w) -> p jc w", w=64)
                for h in range(2):
                    hr0 = 0 if h == 0 else 31
                    jc0 = 1 if h == 0 else 0
                    # kw=1 (center rows): full cells
                    nc.sync.dma_start(
                        s3v[64 * h + 16:64 * h + 32, jc0:jc0 + 33, :],
                        S_d[t, :, hr0:hr0 + 33, :])
                    # kw=0 rows: value S(c, hrow, w-1) at w>=1
                    nc.sync.dma_start(
                        s3v[64 * h:64 * h + 16, jc0:jc0 + 33, 1:64],
                        S_d[t, :, hr0:hr0 + 33, 0:63])
                    # kw=2 rows: value S(c, hrow, w+1) at w<=62
                    nc.sync.dma_start(
                        s3v[64 * h + 32:64 * h + 48, jc0:jc0 + 33, 0:63],
                        S_d[t, :, hr0:hr0 + 33, 1:64])

                # ====== x_pre update + f32r split + bf16 cast ======
                x3o, x3n = (x3fA, x3fB) if t % 2 == 0 else (x3fB, x3fA)
                P3o, P3n = (P3A, P3B) if t % 2 == 0 else (P3B, P3A)
                YIpo, YIpn = (YIpA, YIpB) if t % 2 == 0 else (YIpB, YIpA)
                nc.vector.scalar_tensor_tensor(
                    x3n[:], x3o[:], BETA_S, S3c[:], op.mult, op.add)
                nc.vector.scalar_tensor_tensor(
                    xr[:], x3o[:], BETA_S, S3c[:], op.mult, op.add)
                nc.vector.tensor_tensor(xl[:], x3n[:], xr[:], op.subtract)
                nc.scalar.copy(S3cb[:], S3c[:])

                # ====== conv EARLY: flat chunks, PE sums kh ======
                # W cols stored kh-reversed (khr = 2-kh); moving window for
                # true kh starts at x-col 512*jq + 64*kh.
                isyn = io.tile([128, 1024], f32, tag="isyn")
                for j in range(8):
                    h, jq = j // 4, j % 4
                    g, d = j % 4, j // 4
                    pe = pse.tile([32, 512], f32, tag="pe")
                    first = True
                    for xop in (xr, xl):
                        for kh in (1, 0, 2):
                            c0 = 512 * jq + 64 * kh
                            khr = 2 - kh
                            nc.tensor.matmul(
                                pe[:],
                                Wrs[64 * h:64 * h + 48,
                                    32 * khr:32 * khr + 32],
                                xop[64 * h:64 * h + 48, c0:c0 + 512],
                                start=first, stop=(xop is xl and kh == 2))
                            first = False
                    sl = isyn[32 * g:32 * g + 32, 512 * d:512 * d + 512]
                    if d == 0:
                        nc.vector.tensor_copy(sl, pe[:])
                    else:
                        nc.scalar.copy(sl, pe[:])

                # ====== S3l transposes (bf16) + P3 update ======
                for k0 in range(0, 32, 4):
                    tp = pst.tile([128, 192], bf16, tag="tp")
                    tpv = tp[:].rearrange("p (k c) -> p k c", c=48)
                    for k in range(k0, k0 + 4):
                        h, b = k // 16, k % 16
                        nc.tensor.transpose(
                            tpv[:, k - k0, :],
                            S3cb[64 * h:64 * h + 48, 64 + 128 * b:192 + 128 * b],
                            i48b[64 * h:64 * h + 48, :])
                    nc.scalar.copy(S3l[:, k0:k0 + 4, :], tpv[:])
                nc.vector.scalar_tensor_tensor(
                    P3n[:], P3o[:], BETA_PRE, S3l[:], op.mult, op.add)

                # ====== weight update from AG(t-1), Wfix, late conv ======
                if t > 0:
                    dw8 = io.tile([128, N_CORES, 96], bf16, tag="dw8")
                    gv = cc_out[t - 1].ap().rearrange("r p c -> p r c")
                    nc.sync.dma_start(dw8[:], gv)
                    s4 = io.tile([128, 4, 96], bf16, tag="s4")
                    nc.vector.tensor_tensor(
                        s4[:], dw8[:, 0:4, :], dw8[:, 4:8, :], op.add)
                    s2 = io.tile([128, 2, 96], bf16, tag="s2")
                    nc.vector.tensor_tensor(
                        s2[:], s4[:, 0:2, :], s4[:, 2:4, :], op.add)
                    dws = io.tile([128, 96], f32, tag="dws")
                    nc.vector.tensor_tensor(
                        dws[:], s2[:, 0, :], s2[:, 1, :], op.add)
                    # W += eta_n*(1-W)*dWp ; W *= 1 - eta_n*dWm ; clip
                    u1 = io.tile([48, 96], f32, tag="u1")
                    nc.vector.tensor_scalar(u1[:], Wk[:], -eta_n, eta_n,
                                            op.mult, op.add)
                    nc.vector.tensor_tensor(u1[:], u1[:], dws[0:48, :], op.mult)
                    nc.vector.tensor_tensor(Wk[:], Wk[:], u1[:], op.add)
                    t2 = io.tile([48, 96], f32, tag="t2")
                    nc.vector.tensor_scalar(t2[:], dws[64:112, :], -eta_n, 1.0,
                                            op.mult, op.add)
                    nc.vector.tensor_tensor(Wk[:], Wk[:], t2[:], op.mult)
                    nc.vector.tensor_scalar(Wk[:], Wk[:], 1.0, 0.0,
                                            op.min, op.max)
                # Wfix = W_new - Wrs (f32r); late conv adds Wfix@xr
                nc.vector.tensor_tensor(Wfix[0:48, :], Wk[:], Wrs[0:48, :],
                                        op.subtract)
                nc.vector.tensor_copy(Wfix[64:112, :], Wfix[0:48, :])

                isyn2 = io.tile([128, 1024], f32, tag="isyn2")
                for j in range(8):
                    h, jq = j // 4, j % 4
                    g, d = j % 4, j // 4
                    pl = psl.tile([32, 512], f32, tag="pl")
                    for kh in (1, 0, 2):
                        c0 = 512 * jq + 64 * kh
                        khr = 2 - kh
                        nc.tensor.matmul(
                            pl[:],
                            Wfix[64 * h:64 * h + 48, 32 * khr:32 * khr + 32],
                            xr[64 * h:64 * h + 48, c0:c0 + 512],
                            start=(kh == 1), stop=(kh == 2))
                    sl = isyn[32 * g:32 * g + 32, 512 * d:512 * d + 512]
                    sl2 = isyn2[32 * g:32 * g + 32, 512 * d:512 * d + 512]
                    nc.vector.tensor_tensor(sl2, sl, pl[:], op.add)
                # refresh stale-rounded weights for next step's early conv
                nc.vector.scalar_tensor_tensor(
                    Wrs[0:48, :], Wk[:], 1.0, zero96[:], op.mult, op.add)
                nc.vector.scalar_tensor_tensor(
                    Wrs[64:112, :], Wk[:], 1.0, zero96[:], op.mult, op.add)

                # ====== LIF (fold) ======
                v1 = io.tile([128, 1024], f32, tag="v1")
                v2 = io.tile([128, 1024], f32, tag="v2")
                spk_bA = io.tile([128, 512], bf16, tag="spk_bA")
                spk_bB = io.tile([128, 512], bf16, tag="spk_bB")
                spk_f = io.tile([128, 1024], f32, tag="spk_f")
                for half, spk_h in ((0, spk_bA), (1, spk_bB)):
                    cs = slice(512 * half, 512 * half + 512)
                    nc.vector.scalar_tensor_tensor(
                        v1[:, cs], v[:, cs], BETA_M, isyn2[:, cs],
                        op.mult, op.add)
                    nc.vector.scalar_tensor_tensor(
                        v2[:, cs], ref[:, cs], 0.0, v1[:, cs],
                        op.is_le, op.mult)
                    nc.vector.tensor_scalar(
                        spk_h[:], v2[:, cs], V_TH, None, op.is_ge)
                # tail (off critical path)
                nc.vector.tensor_scalar(
                    spk_f[:], v2[:], V_TH, None, op.is_ge)
                nc.vector.scalar_tensor_tensor(
                    v[:], v2[:], V_TH, v2[:], op.is_lt, op.mult)
                rrelu = io.tile([128, 1024], f32, tag="rrelu")
                nc.vector.tensor_scalar(rrelu[:], ref[:], -1.0, 0.0,
                                        op.add, op.max)
                nc.vector.scalar_tensor_tensor(
                    ref[:], spk_f[:], T_REF, rrelu[:], op.mult, op.add)
                spk8 = io.tile([128, 1024], u8, tag="spk8")
                nc.scalar.copy(spk8[:], spk_f[:])

                # ====== outputs ======
                for dst_d, src in ((spk_d, spk8), (v_d, v), (i_d, isyn2)):
                    dview = dst_d[t].rearrange("o (d g r) -> o d g r",
                                               g=4, r=512)
                    for g in range(4):
                        sview = src[32 * g:32 * g + 32, :].rearrange(
                            "o (d r) -> o d r", r=512)
                        nc.sync.dma_start(dview[:, :, g, :], sview)

                # ====== spk transposes -> YIs odd slots + bulk copies ======
                for d in range(2):
                    for cq in range(4):
                        tq = pst.tile([128, 192], bf16, tag="tp")
                        spk_h = spk_bA if d == 0 else spk_bB
                        nc.tensor.transpose(
                            tq[:, 0:128],
                            spk_h[:, 128 * cq:128 * cq + 128],
                            i128b[:])
                        # chunks k = 16d + 4g + cq -> s = 2k+1, stride 8
                        s0 = 32 * d + 2 * cq + 1
                        dst = YIs[:].rearrange("p s c -> p s c")[
                            :, s0:s0 + 25:8, :]
                        src = tq[:, 0:128].rearrange("p (g c) -> p g c", c=32)
                        nc.scalar.copy(dst, src)
                # bulk even-s fills: YI[p,s,:]=Y(64(s-1)+p)
                nc.vector.tensor_copy(YIs[0:64, 2:65:2, :],
                                      YIs[64:128, 1:64:2, :])
                nc.vector.tensor_copy(YIs[64:128, 0:64:2, :],
                                      YIs[0:64, 1:65:2, :])
                # post-trace update in YI layout (consistent redundancy)
                nc.vector.scalar_tensor_tensor(
                    YIpn[:], YIpo[:], BETA_POST, YIs[:], op.mult, op.add)

                # ====== dW matmuls (bf16, contract 128 l per chunk) ======
                dps = psw.tile([128, 96], f32, tag="dps")
                for k in range(32):
                    nc.tensor.matmul(dps[0:48, :], P3n[:, k, :],
                                     YIs[:, 2 * k:2 * k + 3, :],
                                     start=(k == 0), stop=(k == 31))
                for k in range(32):
                    nc.tensor.matmul(dps[64:112, :], S3l[:, k, :],
                                     YIpn[:, 2 * k:2 * k + 3, :],
                                     start=(k == 0), stop=(k == 31),
                                     tile_position=(0, 64))

                # ====== pack + AllGather (skip on last step) ======
                if t < n_steps - 1:
                    ccs = io.tile([128, 96], bf16, tag="ccs")
                    nc.scalar.copy(ccs[0:48, :], dps[0:48, :])
                    nc.scalar.copy(ccs[64:112, :], dps[64:112, :])
                    nc.sync.dma_start(cc_in[t][:], ccs[:])
                    nc.gpsimd.collective_compute(
                        "AllGather", op.bypass,
                        replica_groups=[list(range(N_CORES))],
                        ins=[cc_in[t].ap().opt()],
                        outs=[cc_out[t].ap().opt()],
                    )

    _split_sync_waits(nc)
    return nc


def kernel(S, W0):
    from concourse import bass_utils

    S = np.ascontiguousarray(np.asarray(S, np.float32))
    W0 = np.asarray(W0, np.float32)
    # rows (kw, c); cols (khr, o) with khr = 2 - kh (reversed so the
    # YI-adjacency dW layout matches W's column layout directly)
    Wk = np.ascontiguousarray(
        W0[:, :, ::-1, :].transpose(3, 1, 2, 0).reshape(48, 96))

    key = T
    if key not in _NC_CACHE:
        _NC_CACHE[key] = _build(T)
    nc = _NC_CACHE[key]

    in_maps = [
        {"S": np.ascontiguousarray(S[:, r]), "Wk": Wk} for r in range(N_CORES)
    ]
    res = bass_utils.run_bass_kernel_spmd(nc, in_maps, core_ids=list(range(N_CORES)))
    global LAST_EXEC_NS, LAST_RES
    LAST_EXEC_NS = getattr(res, "exec_time_ns", None)
    LAST_RES = res

    spikes = np.zeros((T, B, C_OUT, H, W_IN), np.bool_)
    v_traj = np.zeros((T, B, C_OUT, H, W_IN), np.float32)
    i_traj = np.zeros((T, B, C_OUT, H, W_IN), np.float32)
    for r in range(N_CORES):
        o = res.results[r]
        spikes[:, r] = o["spk_out"].reshape(T, C_OUT, H, W_IN) != 0
        v_traj[:, r] = o["v_out"].reshape(T, C_OUT, H, W_IN)
        i_traj[:, r] = o["i_out"].reshape(T, C_OUT, H, W_IN)
    return spikes, v_traj, i_traj
